# revision 17
# baseline (speedup 1.0000x reference)
"""MoE routing + combine kernel for Trainium2, 8 NeuronCores.

Problem: nn_MOELayer (B=2, S=512, H=1024, E=4, V=32000).
The 524MB `logits` tensor dominates; target regime is memory. Tokens
(B*S = 1024) are split 128 per core; each core streams its contiguous
[E,128,V] logits shard and computes out[t,v] = sum_e w[t,e]*lg[e,t,v]
with double-buffered DMA. The router (tiny: [1024,1024]@[1024,4] plus
a softmax/top-p over E=4) runs on host to produce the per-token combine
weights; its global data-dependent max_k couples all tokens, and its
cost is ~0.1% of the combine's memory traffic.

Raw Bass (explicit semaphores): the Tile scheduler's attached waits
trip a 'Too many sync wait commands' codegen limit on this toolchain.
"""

import numpy as np

B, S, H, E, V = 2, 512, 1024, 4, 32000
N = B * S            # 1024 tokens
NCORES = 8
TPC = N // NCORES    # 128 tokens per core
P = 128
VC = 2000            # vocab chunk
NCH = V // VC        # 16 chunks
THRESH = 0.5

_built = None


def _build():
    import concourse.bass as bass
    from concourse import mybir

    f32 = mybir.dt.float32
    OP = mybir.AluOpType

    nc = bass.Bass()
    lg = nc.dram_tensor("lg", [E, TPC, V], f32, kind="ExternalInput")
    wt = nc.dram_tensor("wt", [TPC, E], f32, kind="ExternalInput")
    out = nc.dram_tensor("out", [TPC, V], f32, kind="ExternalOutput")

    with (
        nc.sbuf_tensor([P, 2, E, VC], f32) as lt,     # double-buffered loads
        nc.sbuf_tensor([P, 2, VC], f32) as acc,       # double-buffered output
        nc.sbuf_tensor([P, E], f32) as wsb,
        nc.semaphore("dma_sem") as dma_sem,
        nc.semaphore("v_sem") as v_sem,
        nc.semaphore("st_sem") as st_sem,
        nc.Block() as block,
    ):

        @block.gpsimd
        def _(gpsimd):
            gpsimd.dma_start(out=wsb[:, :], in_=wt[:, :]).then_inc(dma_sem, 16)
            for ci in range(NCH):
                sl = ci % 2
                if ci >= 2:
                    # lt[sl] free once vector finished chunk ci-2
                    gpsimd.wait_ge(v_sem, ci - 1)
                v0 = ci * VC
                for e in range(E):
                    gpsimd.dma_start(
                        out=lt[:, sl, e, :], in_=lg[e, :, v0:v0 + VC]
                    ).then_inc(dma_sem, 16)

        @block.vector
        def _(vector):
            for ci in range(NCH):
                sl = ci % 2
                vector.wait_ge(dma_sem, 16 + (4 * ci + 4) * 16)
                if ci >= 2:
                    # acc[sl] free once store of chunk ci-2 completed
                    vector.wait_ge(st_sem, (ci - 1) * 16)
                nc.vector.tensor_scalar_mul(
                    acc[:, sl, :], lt[:, sl, 0, :], wsb[:, 0:1]
                )
                for e in range(1, E):
                    ins = nc.vector.scalar_tensor_tensor(
                        out=acc[:, sl, :], in0=lt[:, sl, e, :],
                        scalar=wsb[:, e:e + 1], in1=acc[:, sl, :],
                        op0=OP.mult, op1=OP.add,
                    )
                ins.then_inc(v_sem, 1)

        @block.sync
        def _(sync):
            for ci in range(NCH):
                sl = ci % 2
                sync.wait_ge(v_sem, ci + 1)
                sync.dma_start(
                    out=out[:, ci * VC:(ci + 1) * VC], in_=acc[:, sl, :]
                ).then_inc(st_sem, 16)

    return nc


def _get_nc():
    global _built
    if _built is None:
        _built = _build()
    return _built


def host_routing(embedding, noise, Wr, br, Wn, bn):
    """Exact reference semantics in float32 numpy."""
    emb = embedding.reshape(N, H).astype(np.float32)
    rl = emb @ Wr.astype(np.float32) + br.astype(np.float32)
    nl = emb @ Wn.astype(np.float32) + bn.astype(np.float32)
    sp = np.logaddexp(nl, 0.0).astype(np.float32)       # softplus
    rl = rl + noise.reshape(N, E).astype(np.float32) * sp
    m = rl.max(-1, keepdims=True)
    ex = np.exp(rl - m)
    p = (ex / ex.sum(-1, keepdims=True)).astype(np.float32)

    si = np.argsort(-p, axis=-1, kind="stable")
    sprob = np.take_along_axis(p, si, -1)
    cum = np.cumsum(sprob, -1)
    mask = cum < THRESH
    mask[:, 0] = True
    max_k = int(mask.sum(-1).max())
    w = sprob * (np.arange(E) < max_k)
    w = w / (w.sum(-1, keepdims=True) + 1e-6)
    w_orig = np.zeros_like(p)
    np.put_along_axis(w_orig, si, w, -1)
    return w_orig.astype(np.float32), p


def make_in_maps(embedding, logits, noise, Wr, br, Wn, bn):
    w_orig, p = host_routing(embedding, noise, Wr, br, Wn, bn)
    logits = np.ascontiguousarray(logits, dtype=np.float32)
    in_maps = []
    for c in range(NCORES):
        b = c // (S // TPC)
        s0 = (c % (S // TPC)) * TPC
        t0 = c * TPC
        in_maps.append({
            "lg": np.ascontiguousarray(logits[:, b, s0:s0 + TPC, :]),
            "wt": np.ascontiguousarray(w_orig[t0:t0 + TPC, :]),
        })
    return in_maps, p


def run_on_device(in_maps, trace=False):
    from concourse.bass_utils import run_bass_kernel_spmd

    nc = _get_nc()
    return run_bass_kernel_spmd(nc, in_maps, list(range(NCORES)), trace=trace)


def assemble(results, p):
    combined = np.empty((B, S, V), dtype=np.float32)
    for c in range(NCORES):
        b = c // (S // TPC)
        s0 = (c % (S // TPC)) * TPC
        combined[b, s0:s0 + TPC, :] = results[c]["out"]
    route_prob = p.reshape(B, S, E)
    return combined, route_prob, route_prob.copy()


def kernel(embedding, logits, noise, Wr, br, Wn, bn):
    embedding = np.asarray(embedding)
    logits = np.asarray(logits)
    noise = np.asarray(noise)
    Wr, br, Wn, bn = (np.asarray(a) for a in (Wr, br, Wn, bn))
    in_maps, p = make_in_maps(embedding, logits, noise, Wr, br, Wn, bn)
    res = run_on_device(in_maps, trace=False)
    return assemble(res.results, p)


# revision 18
# speedup vs baseline: 1.2741x; 1.2741x over previous
"""MoE routing + combine kernel for Trainium2, 8 NeuronCores.

Problem: nn_MOELayer (B=2, S=512, H=1024, E=4, V=32000).
The 524MB `logits` tensor dominates; target regime is memory. Tokens
(B*S = 1024) are split 128 per core; each core streams its contiguous
[E,128,V] logits shard and computes out[t,v] = sum_e w[t,e]*lg[e,t,v]
with double-buffered DMA. The router (tiny: [1024,1024]@[1024,4] plus
a softmax/top-p over E=4) runs on host to produce the per-token combine
weights; its global data-dependent max_k couples all tokens, and its
cost is ~0.1% of the combine's memory traffic.

Raw Bass (explicit semaphores): the Tile scheduler's attached waits
trip a 'Too many sync wait commands' codegen limit on this toolchain.
"""

import numpy as np

B, S, H, E, V = 2, 512, 1024, 4, 32000
N = B * S            # 1024 tokens
NCORES = 8
TPC = N // NCORES    # 128 tokens per core
P = 128
VC = 4000             # vocab chunk
NCH = V // VC        # 16 chunks
THRESH = 0.5

_built = None


def _build():
    import concourse.bass as bass
    from concourse import mybir

    f32 = mybir.dt.float32
    OP = mybir.AluOpType

    nc = bass.Bass()
    lg = nc.dram_tensor("lg", [E, TPC, V], f32, kind="ExternalInput")
    wt = nc.dram_tensor("wt", [TPC, E], f32, kind="ExternalInput")
    out = nc.dram_tensor("out", [TPC, V], f32, kind="ExternalOutput")

    with (
        nc.sbuf_tensor([P, 2, E, VC], f32) as lt,     # double-buffered loads
        nc.sbuf_tensor([P, 2, VC], f32) as acc,       # double-buffered output
        nc.sbuf_tensor([P, E], f32) as wsb,
        nc.semaphore("dma_sem") as dma_sem,
        nc.semaphore("v_sem") as v_sem,
        nc.semaphore("st_sem") as st_sem,
        nc.Block() as block,
    ):

        @block.gpsimd
        def _(gpsimd):
            gpsimd.dma_start(out=wsb[:, :], in_=wt[:, :]).then_inc(dma_sem, 16)
            for ci in range(NCH):
                sl = ci % 2
                if ci >= 2:
                    # lt[sl] free once vector finished chunk ci-2
                    gpsimd.wait_ge(v_sem, ci - 1)
                v0 = ci * VC
                for e in range(E):
                    gpsimd.dma_start(
                        out=lt[:, sl, e, :], in_=lg[e, :, v0:v0 + VC]
                    ).then_inc(dma_sem, 16)

        @block.vector
        def _(vector):
            for ci in range(NCH):
                sl = ci % 2
                vector.wait_ge(dma_sem, 16 + (4 * ci + 4) * 16)
                if ci >= 2:
                    # acc[sl] free once store of chunk ci-2 completed
                    vector.wait_ge(st_sem, (ci - 1) * 16)
                nc.vector.tensor_scalar_mul(
                    acc[:, sl, :], lt[:, sl, 0, :], wsb[:, 0:1]
                )
                for e in range(1, E):
                    ins = nc.vector.scalar_tensor_tensor(
                        out=acc[:, sl, :], in0=lt[:, sl, e, :],
                        scalar=wsb[:, e:e + 1], in1=acc[:, sl, :],
                        op0=OP.mult, op1=OP.add,
                    )
                ins.then_inc(v_sem, 1)

        @block.sync
        def _(sync):
            for ci in range(NCH):
                sl = ci % 2
                sync.wait_ge(v_sem, ci + 1)
                sync.dma_start(
                    out=out[:, ci * VC:(ci + 1) * VC], in_=acc[:, sl, :]
                ).then_inc(st_sem, 16)

    return nc


def _get_nc():
    global _built
    if _built is None:
        _built = _build()
    return _built


def host_routing(embedding, noise, Wr, br, Wn, bn):
    """Exact reference semantics in float32 numpy."""
    emb = embedding.reshape(N, H).astype(np.float32)
    rl = emb @ Wr.astype(np.float32) + br.astype(np.float32)
    nl = emb @ Wn.astype(np.float32) + bn.astype(np.float32)
    sp = np.logaddexp(nl, 0.0).astype(np.float32)       # softplus
    rl = rl + noise.reshape(N, E).astype(np.float32) * sp
    m = rl.max(-1, keepdims=True)
    ex = np.exp(rl - m)
    p = (ex / ex.sum(-1, keepdims=True)).astype(np.float32)

    si = np.argsort(-p, axis=-1, kind="stable")
    sprob = np.take_along_axis(p, si, -1)
    cum = np.cumsum(sprob, -1)
    mask = cum < THRESH
    mask[:, 0] = True
    max_k = int(mask.sum(-1).max())
    w = sprob * (np.arange(E) < max_k)
    w = w / (w.sum(-1, keepdims=True) + 1e-6)
    w_orig = np.zeros_like(p)
    np.put_along_axis(w_orig, si, w, -1)
    return w_orig.astype(np.float32), p


def make_in_maps(embedding, logits, noise, Wr, br, Wn, bn):
    w_orig, p = host_routing(embedding, noise, Wr, br, Wn, bn)
    logits = np.ascontiguousarray(logits, dtype=np.float32)
    in_maps = []
    for c in range(NCORES):
        b = c // (S // TPC)
        s0 = (c % (S // TPC)) * TPC
        t0 = c * TPC
        in_maps.append({
            "lg": np.ascontiguousarray(logits[:, b, s0:s0 + TPC, :]),
            "wt": np.ascontiguousarray(w_orig[t0:t0 + TPC, :]),
        })
    return in_maps, p


def run_on_device(in_maps, trace=False):
    from concourse.bass_utils import run_bass_kernel_spmd

    nc = _get_nc()
    return run_bass_kernel_spmd(nc, in_maps, list(range(NCORES)), trace=trace)


def assemble(results, p):
    combined = np.empty((B, S, V), dtype=np.float32)
    for c in range(NCORES):
        b = c // (S // TPC)
        s0 = (c % (S // TPC)) * TPC
        combined[b, s0:s0 + TPC, :] = results[c]["out"]
    route_prob = p.reshape(B, S, E)
    return combined, route_prob, route_prob.copy()


def kernel(embedding, logits, noise, Wr, br, Wn, bn):
    embedding = np.asarray(embedding)
    logits = np.asarray(logits)
    noise = np.asarray(noise)
    Wr, br, Wn, bn = (np.asarray(a) for a in (Wr, br, Wn, bn))
    in_maps, p = make_in_maps(embedding, logits, noise, Wr, br, Wn, bn)
    res = run_on_device(in_maps, trace=False)
    return assemble(res.results, p)
